# revision 4
# baseline (speedup 1.0000x reference)
"""2-layer LSTM (S=512, B=64, I=H=1024) on 8 Trainium2 NeuronCores.

Strategy: tensor-parallel over the 4H gate dimension. Each core owns a
128-column slice of each of the i/f/o/g gates per layer (512 gate columns
total per layer). Per time step every core computes its slice of the gates,
the corresponding h-chunk, and the 8 h^T chunks are exchanged with an
AllGather so every core has the full h_t for the next step's matmuls.

Layer 1 is interleaved one step behind layer 0 so a single merged AllGather
per step carries both layers' chunks. The x @ Wih0 part for all timesteps is
precomputed as a column-sharded GEMM (phase A). Layer 1's input matmul
(h0 @ Wih1) is folded into the per-step recurrence matmuls.

Matmuls run in float32r (TF32-like, full PE rate); state stays float32.
"""
import sys

sys.path.insert(0, "/opt/trn_rl_repo")

import numpy as np

S, B, I, H = 512, 64, 1024, 1024
NC = 8            # cores
HC = H // NC      # h-chunk columns per core (128)
GC = 4 * HC       # gate columns per core (512)
KT = H // 128     # K tiles (8)

_cache = {}


def _build(s):
    import concourse.bass as bass
    import concourse.bacc as bacc
    import concourse.mybir as mybir
    import concourse.tile as tile

    f32 = mybir.dt.float32
    f32r = mybir.dt.float32r
    ADD = mybir.AluOpType.add
    MULT = mybir.AluOpType.mult
    SIG = mybir.ActivationFunctionType.Sigmoid
    TANH = mybir.ActivationFunctionType.Tanh

    chunk = s * B // NC        # per-core (t,b) columns of x^T
    n_mt = s * B // 128        # phase-A M tiles

    nc = bacc.Bacc("TRN2", target_bir_lowering=False, debug=False, num_devices=NC)

    xT = nc.dram_tensor("xT", [I, chunk], f32, kind="ExternalInput")
    wih0 = nc.dram_tensor("wih0", [I, GC], f32, kind="ExternalInput")
    whh0 = nc.dram_tensor("whh0", [H, GC], f32, kind="ExternalInput")
    wih1 = nc.dram_tensor("wih1", [H, GC], f32, kind="ExternalInput")
    whh1 = nc.dram_tensor("whh1", [H, GC], f32, kind="ExternalInput")
    b0 = nc.dram_tensor("b0", [128, GC], f32, kind="ExternalInput")
    b1 = nc.dram_tensor("b1", [B, GC], f32, kind="ExternalInput")
    ident = nc.dram_tensor("ident", [B, B], f32, kind="ExternalInput")

    out_c = nc.dram_tensor("out_c", [s, B, HC], f32, kind="ExternalOutput")
    hn_c = nc.dram_tensor("hn_c", [2, B, HC], f32, kind="ExternalOutput")
    cn_c = nc.dram_tensor("cn_c", [2, B, HC], f32, kind="ExternalOutput")

    rg = [list(range(NC))]

    with tile.TileContext(nc) as tc:
        with tc.tile_pool(name="const", bufs=1) as cpool, \
             tc.tile_pool(name="wpool", bufs=1) as wpool, \
             tc.tile_pool(name="dram", bufs=1, space="DRAM") as dram, \
             tc.tile_pool(name="sbuf", bufs=4) as spool, \
             tc.tile_pool(name="xpool", bufs=2) as xpool, \
             tc.tile_pool(name="hpool", bufs=3) as hpool, \
             tc.tile_pool(name="stg", bufs=2) as stg, \
             tc.tile_pool(name="dbounce", bufs=2, space="DRAM") as dbounce:

            # ---- gather x^T from all cores --------------------------------
            xin = dram.tile([I, chunk], f32)
            xall = dram.tile([NC * I, chunk], f32)   # [time-chunk r][I, chunk]
            nc.sync.dma_start(out=xin[:], in_=xT[:])
            nc.gpsimd.collective_compute(
                "AllGather", mybir.AluOpType.bypass,
                ins=[xin.opt()], outs=[xall.opt()], replica_groups=rg,
            )

            # ---- constants ------------------------------------------------
            idt = cpool.tile([B, B], f32)
            nc.sync.dma_start(out=idt[:], in_=ident[:])
            b0t = cpool.tile([128, GC], f32)
            nc.sync.dma_start(out=b0t[:], in_=b0[:])
            b1t = cpool.tile([B, GC], f32)
            nc.sync.dma_start(out=b1t[:], in_=b1[:])

            # ---- weights: load fp32, round to fp32r -----------------------
            wsb = {}
            for name, t in (("wih0", wih0), ("whh0", whh0),
                            ("wih1", wih1), ("whh1", whh1)):
                tmp = xpool.tile([128, KT * GC], f32, tag="xstage")
                nc.sync.dma_start(
                    out=tmp[:].rearrange("p (k n) -> p k n", k=KT),
                    in_=t[:].rearrange("(k p) n -> p k n", p=128))
                wr = wpool.tile([128, KT * GC], f32r, tag=name)
                nc.vector.tensor_copy(wr[:], tmp[:])
                wsb[name] = wr

            g0_dram = dram.tile([s * B, GC], f32)

            # ---- phase A: G0 = x @ Wih0 + b0 ------------------------------
            with tc.tile_pool(name="psumA", bufs=3, space="PSUM") as psA:
                for m in range(n_mt):
                    r = (m * 128) // chunk
                    col0 = (m * 128) % chunk
                    xt = xpool.tile([128, KT * 128], f32, tag="xstage")
                    nc.sync.dma_start(
                        out=xt[:].rearrange("p (k j) -> p k j", k=KT),
                        in_=xall[r * I:(r + 1) * I, col0:col0 + 128]
                        .rearrange("(k p) j -> p k j", p=128))
                    xtr = xpool.tile([128, KT * 128], f32r, tag="xtr")
                    nc.vector.tensor_copy(xtr[:], xt[:])
                    ps = psA.tile([128, GC], f32, tag="psA")
                    for k in range(KT):
                        nc.tensor.matmul(
                            ps[:], xtr[:, k * 128:(k + 1) * 128],
                            wsb["wih0"][:, k * GC:(k + 1) * GC],
                            start=(k == 0), stop=(k == KT - 1))
                    gsb = spool.tile([128, GC], f32, tag="gA")
                    nc.vector.tensor_tensor(out=gsb[:], in0=ps[:], in1=b0t[:], op=ADD)
                    nc.sync.dma_start(out=g0_dram[m * 128:(m + 1) * 128, :], in_=gsb[:])

            # ---- recurrence ----------------------------------------------
            with tc.tile_pool(name="psumR", bufs=2, space="PSUM") as psR:

                def half_step(layer, gates_ps, addend, c_prev):
                    """gates_ps [B, GC] + addend -> (h, c_new) [B, HC] f32."""
                    L = str(layer)
                    gsb = spool.tile([B, GC], f32, tag="g" + L)
                    nc.vector.tensor_tensor(out=gsb[:], in0=gates_ps[:], in1=addend, op=ADD)
                    sig = spool.tile([B, 3 * HC], f32, tag="s" + L)
                    nc.scalar.activation(sig[:], gsb[:, 0:3 * HC], SIG)
                    gt = spool.tile([B, HC], f32, tag="gt" + L)
                    nc.scalar.activation(gt[:], gsb[:, 3 * HC:4 * HC], TANH)
                    t1 = spool.tile([B, HC], f32, tag="t1" + L)
                    nc.vector.tensor_tensor(out=t1[:], in0=sig[:, HC:2 * HC], in1=c_prev[:], op=MULT)
                    t2 = spool.tile([B, HC], f32, tag="t2" + L)
                    nc.vector.tensor_tensor(out=t2[:], in0=sig[:, 0:HC], in1=gt[:], op=MULT)
                    c_new = spool.tile([B, HC], f32, tag="c" + L)
                    nc.vector.tensor_tensor(out=c_new[:], in0=t1[:], in1=t2[:], op=ADD)
                    tc_ = spool.tile([B, HC], f32, tag="tc" + L)
                    nc.scalar.activation(tc_[:], c_new[:], TANH)
                    h = spool.tile([B, HC], f32, tag="h" + L)
                    nc.vector.tensor_tensor(out=h[:], in0=sig[:, 2 * HC:3 * HC], in1=tc_[:], op=MULT)
                    return h, c_new

                hT_prev = hpool.tile([128, 2 * NC * B], f32r, tag="hT")
                nc.vector.memset(hT_prev[:].bitcast(f32), 0.0)
                c0 = spool.tile([B, HC], f32, tag="c0")
                nc.vector.memset(c0[:], 0.0)
                c1 = spool.tile([B, HC], f32, tag="c1")
                nc.vector.memset(c1[:], 0.0)
                h0f = h1f = None

                for t in range(s):
                    # layer-0 step t
                    ps0 = psR.tile([B, GC], f32, tag="ps0")
                    for k in range(KT):
                        nc.tensor.matmul(
                            ps0[:], hT_prev[:, k * B:(k + 1) * B],
                            wsb["whh0"][:, k * GC:(k + 1) * GC],
                            start=(k == 0), stop=(k == KT - 1))
                    g0t = spool.tile([B, GC], f32, tag="g0t")
                    nc.sync.dma_start(out=g0t[:], in_=g0_dram[t * B:(t + 1) * B, :])
                    h0, c0 = half_step(0, ps0, g0t[:], c0)

                    # layer-1 step t-1
                    if t >= 1:
                        ps1 = psR.tile([B, GC], f32, tag="ps1")
                        for k in range(KT):
                            nc.tensor.matmul(
                                ps1[:], hT_prev[:, k * B:(k + 1) * B],
                                wsb["wih1"][:, k * GC:(k + 1) * GC],
                                start=(k == 0), stop=False)
                        for k in range(KT):
                            nc.tensor.matmul(
                                ps1[:], hT_prev[:, (KT + k) * B:(KT + k + 1) * B],
                                wsb["whh1"][:, k * GC:(k + 1) * GC],
                                start=False, stop=(k == KT - 1))
                        h1, c1 = half_step(1, ps1, b1t[:], c1)
                        nc.sync.dma_start(out=out_c[t - 1], in_=h1[:])
                        h1f = h1

                    # stage h0^T(t) | h1^T(t-1) and AllGather
                    stage = stg.tile([128, 2 * B], f32r, tag="stage")
                    tp0 = psR.tile([128, B], f32, tag="tp")
                    nc.tensor.transpose(tp0[:], h0[:], idt[:])
                    nc.vector.tensor_copy(stage[:, 0:B], tp0[:])
                    if t >= 1:
                        tp1 = psR.tile([128, B], f32, tag="tp")
                        nc.tensor.transpose(tp1[:], h1[:], idt[:])
                        nc.vector.tensor_copy(stage[:, B:2 * B], tp1[:])
                    else:
                        nc.vector.memset(stage[:, B:2 * B].bitcast(f32), 0.0)

                    if t < s - 1:
                        bin_t = dbounce.tile([128, 2 * B], f32r, tag="bin")
                        bout_t = dbounce.tile([NC * 128, 2 * B], f32r, tag="bout")
                        nc.sync.dma_start(out=bin_t[:], in_=stage[:])
                        nc.gpsimd.collective_compute(
                            "AllGather", mybir.AluOpType.bypass,
                            ins=[bin_t.opt()], outs=[bout_t.opt()], replica_groups=rg)
                        hT_t = hpool.tile([128, 2 * NC * B], f32r, tag="hT")
                        for g in range(2):
                            nc.sync.dma_start(
                                out=hT_t[:, g * NC * B:(g + 1) * NC * B]
                                .rearrange("p (r j) -> p r j", r=NC),
                                in_=bout_t[:, g * B:(g + 1) * B]
                                .rearrange("(r p) j -> p r j", p=128))
                        hT_prev = hT_t
                    h0f = h0

                # final layer-1 step (t = s-1): needs full h0(s-1) -> use last AG?
                # h0(s-1) was not gathered (t == s-1 skipped AG). Gather it now.
                bin_t = dbounce.tile([128, 2 * B], f32r, tag="bin")
                bout_t = dbounce.tile([NC * 128, 2 * B], f32r, tag="bout")
                nc.sync.dma_start(out=bin_t[:], in_=stage[:])
                nc.gpsimd.collective_compute(
                    "AllGather", mybir.AluOpType.bypass,
                    ins=[bin_t.opt()], outs=[bout_t.opt()], replica_groups=rg)
                hT_t = hpool.tile([128, 2 * NC * B], f32r, tag="hT")
                for g in range(2):
                    nc.sync.dma_start(
                        out=hT_t[:, g * NC * B:(g + 1) * NC * B]
                        .rearrange("p (r j) -> p r j", r=NC),
                        in_=bout_t[:, g * B:(g + 1) * B]
                        .rearrange("(r p) j -> p r j", p=128))
                hT_prev = hT_t

                ps1 = psR.tile([B, GC], f32, tag="ps1")
                for k in range(KT):
                    nc.tensor.matmul(
                        ps1[:], hT_prev[:, k * B:(k + 1) * B],
                        wsb["wih1"][:, k * GC:(k + 1) * GC],
                        start=(k == 0), stop=False)
                for k in range(KT):
                    nc.tensor.matmul(
                        ps1[:], hT_prev[:, (KT + k) * B:(KT + k + 1) * B],
                        wsb["whh1"][:, k * GC:(k + 1) * GC],
                        start=False, stop=(k == KT - 1))
                h1, c1 = half_step(1, ps1, b1t[:], c1)
                nc.sync.dma_start(out=out_c[s - 1], in_=h1[:])

                nc.sync.dma_start(out=hn_c[0], in_=h0f[:])
                nc.sync.dma_start(out=hn_c[1], in_=h1[:])
                nc.sync.dma_start(out=cn_c[0], in_=c0[:])
                nc.sync.dma_start(out=cn_c[1], in_=c1[:])

    nc.compile()
    return nc


def _shard_inputs(s, x, Wih0, bih0, Whh0, bhh0, Wih1, bih1, Whh1, bhh1):
    xt = np.ascontiguousarray(x.reshape(s * B, I).T)      # [I, S*B]
    chunk = s * B // NC
    ident = np.eye(B, dtype=np.float32)
    in_maps = []
    for c in range(NC):
        cols = np.concatenate(
            [g * H + c * HC + np.arange(HC) for g in range(4)])
        b0v = (np.asarray(bih0) + np.asarray(bhh0))[cols].astype(np.float32)
        b1v = (np.asarray(bih1) + np.asarray(bhh1))[cols].astype(np.float32)
        in_maps.append({
            "xT": np.ascontiguousarray(xt[:, c * chunk:(c + 1) * chunk]),
            "wih0": np.ascontiguousarray(np.asarray(Wih0)[:, cols]),
            "whh0": np.ascontiguousarray(np.asarray(Whh0)[:, cols]),
            "wih1": np.ascontiguousarray(np.asarray(Wih1)[:, cols]),
            "whh1": np.ascontiguousarray(np.asarray(Whh1)[:, cols]),
            "b0": np.ascontiguousarray(np.broadcast_to(b0v, (128, GC))),
            "b1": np.ascontiguousarray(np.broadcast_to(b1v, (B, GC))),
            "ident": ident,
        })
    return in_maps


def _run(s, in_maps, trace=False):
    from concourse import bass_utils
    key = (s,)
    if key not in _cache:
        _cache[key] = _build(s)
    nc = _cache[key]
    res = bass_utils.run_bass_kernel_spmd(
        nc, in_maps, core_ids=list(range(NC)), trace=trace)
    return res


def _assemble(s, results):
    output = np.empty((s, B, H), np.float32)
    h_n = np.empty((2, B, H), np.float32)
    c_n = np.empty((2, B, H), np.float32)
    for c in range(NC):
        sl = slice(c * HC, (c + 1) * HC)
        output[:, :, sl] = results[c]["out_c"]
        h_n[:, :, sl] = results[c]["hn_c"]
        c_n[:, :, sl] = results[c]["cn_c"]
    return output, h_n, c_n


def kernel(x, Wih0, bih0, Whh0, bhh0, Wih1, bih1, Whh1, bhh1):
    x = np.asarray(x, np.float32)
    in_maps = _shard_inputs(S, x, Wih0, bih0, Whh0, bhh0,
                            Wih1, bih1, Whh1, bhh1)
    res = _run(S, in_maps)
    return _assemble(S, res.results)


# revision 9
# speedup vs baseline: 194.6387x; 194.6387x over previous
"""2-layer LSTM (S=512, B=64, I=H=1024) on 8 Trainium2 NeuronCores.

Strategy: tensor-parallel over the 4H gate dimension. Each core owns a
128-column slice of each of the i/f/o/g gates per layer (512 gate columns
total per layer). Per time step every core computes its slice of the gates,
the corresponding h-chunk, and the 8 h^T chunks are exchanged with an
AllGather so every core has the full h_t for the next step's matmuls.

Layer 1 is interleaved one step behind layer 0 so a single merged AllGather
per step carries both layers' chunks. The x @ Wih0 part for all timesteps is
precomputed as a column-sharded GEMM (phase A). Layer 1's input matmul
(h0 @ Wih1) is folded into the per-step recurrence matmuls.

Matmuls run in float32r (TF32-like, full PE rate); state stays float32.
"""
import sys

sys.path.insert(0, "/opt/trn_rl_repo")

import numpy as np

S, B, I, H = 512, 64, 1024, 1024
NC = 8            # cores
HC = H // NC      # h-chunk columns per core (128)
GC = 4 * HC       # gate columns per core (512)
KT = H // 128     # K tiles (8)

_cache = {}


def _build(s):
    import concourse.bass as bass
    import concourse.bacc as bacc
    import concourse.mybir as mybir
    import concourse.tile as tile

    f32 = mybir.dt.float32
    f32r = mybir.dt.float32r
    ADD = mybir.AluOpType.add
    MULT = mybir.AluOpType.mult
    SIG = mybir.ActivationFunctionType.Sigmoid
    TANH = mybir.ActivationFunctionType.Tanh

    chunk = s * B // NC        # per-core (t,b) columns of x^T
    n_mt = s * B // 128        # phase-A M tiles

    nc = bacc.Bacc("TRN2", target_bir_lowering=False, debug=False, num_devices=NC)

    xT = nc.dram_tensor("xT", [I, chunk], f32, kind="ExternalInput")
    wih0 = nc.dram_tensor("wih0", [I, GC], f32, kind="ExternalInput")
    whh0 = nc.dram_tensor("whh0", [H, GC], f32, kind="ExternalInput")
    wih1 = nc.dram_tensor("wih1", [H, GC], f32, kind="ExternalInput")
    whh1 = nc.dram_tensor("whh1", [H, GC], f32, kind="ExternalInput")
    b0 = nc.dram_tensor("b0", [128, GC], f32, kind="ExternalInput")
    b1 = nc.dram_tensor("b1", [B, GC], f32, kind="ExternalInput")
    ident = nc.dram_tensor("ident", [B, B], f32, kind="ExternalInput")

    out_c = nc.dram_tensor("out_c", [s, B, HC], f32, kind="ExternalOutput")
    hn_c = nc.dram_tensor("hn_c", [2, B, HC], f32, kind="ExternalOutput")
    cn_c = nc.dram_tensor("cn_c", [2, B, HC], f32, kind="ExternalOutput")

    rg = [list(range(NC))]

    with tile.TileContext(nc) as tc:
        with tc.tile_pool(name="const", bufs=1) as cpool, \
             tc.tile_pool(name="wpool", bufs=1) as wpool, \
             tc.tile_pool(name="dram", bufs=1, space="DRAM") as dram, \
             tc.tile_pool(name="sbuf", bufs=4) as spool, \
             tc.tile_pool(name="xpool", bufs=2) as xpool, \
             tc.tile_pool(name="hpool", bufs=3) as hpool, \
             tc.tile_pool(name="stg", bufs=2) as stg, \
             tc.tile_pool(name="dbounce", bufs=2, space="DRAM") as dbounce:

            # ---- gather x^T from all cores --------------------------------
            xin = dram.tile([I, chunk], f32)
            xall = dram.tile([NC * I, chunk], f32, addr_space="Shared")   # [time-chunk r][I, chunk]
            nc.sync.dma_start(out=xin[:], in_=xT[:])
            nc.gpsimd.collective_compute(
                "AllGather", mybir.AluOpType.bypass,
                ins=[xin.opt()], outs=[xall.opt()], replica_groups=rg,
            )

            # ---- constants ------------------------------------------------
            idt = cpool.tile([B, B], f32)
            nc.sync.dma_start(out=idt[:], in_=ident[:])
            b0t = cpool.tile([128, GC], f32)
            nc.sync.dma_start(out=b0t[:], in_=b0[:])
            b1t = cpool.tile([B, GC], f32)
            nc.sync.dma_start(out=b1t[:], in_=b1[:])

            # ---- weights: load fp32, round to fp32r -----------------------
            wsb = {}
            for name, t in (("wih0", wih0), ("whh0", whh0),
                            ("wih1", wih1), ("whh1", whh1)):
                tmp = xpool.tile([128, KT * GC], f32, tag="xstage")
                nc.sync.dma_start(
                    out=tmp[:].rearrange("p (k n) -> p k n", k=KT),
                    in_=t[:].rearrange("(k p) n -> p k n", p=128))
                wr = wpool.tile([128, KT * GC], f32r, tag=name)
                nc.vector.tensor_copy(wr[:], tmp[:])
                wsb[name] = wr

            g0_dram = dram.tile([s * B, GC], f32)

            # ---- phase A: G0 = x @ Wih0 + b0 ------------------------------
            with tc.tile_pool(name="psumA", bufs=3, space="PSUM") as psA:
                for m in range(n_mt):
                    r = (m * 128) // chunk
                    col0 = (m * 128) % chunk
                    xt = xpool.tile([128, KT * 128], f32, tag="xstage")
                    nc.sync.dma_start(
                        out=xt[:].rearrange("p (k j) -> p k j", k=KT),
                        in_=xall[r * I:(r + 1) * I, col0:col0 + 128]
                        .rearrange("(k p) j -> p k j", p=128))
                    xtr = xpool.tile([128, KT * 128], f32r, tag="xtr")
                    nc.vector.tensor_copy(xtr[:], xt[:])
                    ps = psA.tile([128, GC], f32, tag="psA")
                    for k in range(KT):
                        nc.tensor.matmul(
                            ps[:], xtr[:, k * 128:(k + 1) * 128],
                            wsb["wih0"][:, k * GC:(k + 1) * GC],
                            start=(k == 0), stop=(k == KT - 1))
                    gsb = spool.tile([128, GC], f32, tag="gA")
                    nc.vector.tensor_tensor(out=gsb[:], in0=ps[:], in1=b0t[:], op=ADD)
                    nc.sync.dma_start(out=g0_dram[m * 128:(m + 1) * 128, :], in_=gsb[:])

            # ---- recurrence ----------------------------------------------
            with tc.tile_pool(name="psumR", bufs=2, space="PSUM") as psR:

                def half_step(layer, gates_ps, addend, c_prev):
                    """gates_ps [B, GC] + addend -> (h, c_new) [B, HC] f32."""
                    L = str(layer)
                    gsb = spool.tile([B, GC], f32, tag="g" + L)
                    nc.vector.tensor_tensor(out=gsb[:], in0=gates_ps[:], in1=addend, op=ADD)
                    sig = spool.tile([B, 3 * HC], f32, tag="s" + L)
                    nc.scalar.activation(sig[:], gsb[:, 0:3 * HC], SIG)
                    gt = spool.tile([B, HC], f32, tag="gt" + L)
                    nc.scalar.activation(gt[:], gsb[:, 3 * HC:4 * HC], TANH)
                    t1 = spool.tile([B, HC], f32, tag="t1" + L)
                    nc.vector.tensor_tensor(out=t1[:], in0=sig[:, HC:2 * HC], in1=c_prev[:], op=MULT)
                    t2 = spool.tile([B, HC], f32, tag="t2" + L)
                    nc.vector.tensor_tensor(out=t2[:], in0=sig[:, 0:HC], in1=gt[:], op=MULT)
                    c_new = spool.tile([B, HC], f32, tag="c" + L)
                    nc.vector.tensor_tensor(out=c_new[:], in0=t1[:], in1=t2[:], op=ADD)
                    tc_ = spool.tile([B, HC], f32, tag="tc" + L)
                    nc.scalar.activation(tc_[:], c_new[:], TANH)
                    h = spool.tile([B, HC], f32, tag="h" + L)
                    nc.vector.tensor_tensor(out=h[:], in0=sig[:, 2 * HC:3 * HC], in1=tc_[:], op=MULT)
                    return h, c_new

                def gather(h_tile, tag):
                    """Transpose h chunk, AllGather, return hT tile [128, NC*B]."""
                    tp = psR.tile([128, B], f32, tag="tp")
                    nc.tensor.transpose(tp[:], h_tile[:], idt[:])
                    stage = stg.tile([128, B], f32r, tag="stg" + tag)
                    nc.vector.tensor_copy(stage[:], tp[:])
                    bin_t = dbounce.tile([128, B], f32r, tag="bin" + tag)
                    bout_t = dbounce.tile([NC * 128, B], f32r, tag="bout" + tag, addr_space="Shared")
                    nc.sync.dma_start(out=bin_t[:], in_=stage[:])
                    nc.gpsimd.collective_compute(
                        "AllGather", mybir.AluOpType.bypass,
                        ins=[bin_t.opt()], outs=[bout_t.opt()], replica_groups=rg)
                    hT_t = hpool.tile([128, NC * B], f32r, tag="hT" + tag)
                    nc.sync.dma_start(
                        out=hT_t[:].rearrange("p (r j) -> p r j", r=NC),
                        in_=bout_t[:].rearrange("(r p) j -> p r j", p=128))
                    return hT_t

                hT0_prev = hpool.tile([128, NC * B], f32r, tag="hT0")
                nc.vector.memset(hT0_prev[:].bitcast(f32), 0.0)
                hT1_prev = hpool.tile([128, NC * B], f32r, tag="hT1")
                nc.vector.memset(hT1_prev[:].bitcast(f32), 0.0)
                c0 = spool.tile([B, HC], f32, tag="c0")
                nc.vector.memset(c0[:], 0.0)
                c1 = spool.tile([B, HC], f32, tag="c1")
                nc.vector.memset(c1[:], 0.0)
                h0f = h1f = None

                def l1_step(hT0_in, hT1_in, c1_in):
                    ps1 = psR.tile([B, GC], f32, tag="ps1")
                    for k in range(KT):
                        nc.tensor.matmul(
                            ps1[:], hT0_in[:, k * B:(k + 1) * B],
                            wsb["wih1"][:, k * GC:(k + 1) * GC],
                            start=(k == 0), stop=False)
                    for k in range(KT):
                        nc.tensor.matmul(
                            ps1[:], hT1_in[:, k * B:(k + 1) * B],
                            wsb["whh1"][:, k * GC:(k + 1) * GC],
                            start=False, stop=(k == KT - 1))
                    return half_step(1, ps1, b1t[:], c1_in)

                for t in range(s):
                    hT0_old = hT0_prev
                    # layer-0 step t
                    ps0 = psR.tile([B, GC], f32, tag="ps0")
                    for k in range(KT):
                        nc.tensor.matmul(
                            ps0[:], hT0_prev[:, k * B:(k + 1) * B],
                            wsb["whh0"][:, k * GC:(k + 1) * GC],
                            start=(k == 0), stop=(k == KT - 1))
                    g0t = spool.tile([B, GC], f32, tag="g0t")
                    nc.sync.dma_start(out=g0t[:], in_=g0_dram[t * B:(t + 1) * B, :])
                    h0, c0 = half_step(0, ps0, g0t[:], c0)
                    if t < s - 1:
                        hT0_prev = gather(h0, "0")

                    # layer-1 step t-1 (overlaps AG0(t))
                    if t >= 1:
                        h1, c1 = l1_step(hT0_old, hT1_prev, c1)
                        nc.sync.dma_start(out=out_c[t - 1], in_=h1[:])
                        h1f = h1
                        hT1_prev = gather(h1, "1")
                    h0f = h0

                # final layer-1 step t = s-1: needs full h0(s-1)
                hT0_prev = gather(h0f, "0")
                h1, c1 = l1_step(hT0_prev, hT1_prev, c1)
                nc.sync.dma_start(out=out_c[s - 1], in_=h1[:])

                nc.sync.dma_start(out=hn_c[0], in_=h0f[:])
                nc.sync.dma_start(out=hn_c[1], in_=h1[:])
                nc.sync.dma_start(out=cn_c[0], in_=c0[:])
                nc.sync.dma_start(out=cn_c[1], in_=c1[:])

    nc.compile()
    return nc


def _shard_inputs(s, x, Wih0, bih0, Whh0, bhh0, Wih1, bih1, Whh1, bhh1):
    xt = np.ascontiguousarray(x.reshape(s * B, I).T)      # [I, S*B]
    chunk = s * B // NC
    ident = np.eye(B, dtype=np.float32)
    in_maps = []
    for c in range(NC):
        cols = np.concatenate(
            [g * H + c * HC + np.arange(HC) for g in range(4)])
        b0v = (np.asarray(bih0) + np.asarray(bhh0))[cols].astype(np.float32)
        b1v = (np.asarray(bih1) + np.asarray(bhh1))[cols].astype(np.float32)
        in_maps.append({
            "xT": np.ascontiguousarray(xt[:, c * chunk:(c + 1) * chunk]),
            "wih0": np.ascontiguousarray(np.asarray(Wih0)[:, cols]),
            "whh0": np.ascontiguousarray(np.asarray(Whh0)[:, cols]),
            "wih1": np.ascontiguousarray(np.asarray(Wih1)[:, cols]),
            "whh1": np.ascontiguousarray(np.asarray(Whh1)[:, cols]),
            "b0": np.ascontiguousarray(np.broadcast_to(b0v, (128, GC))),
            "b1": np.ascontiguousarray(np.broadcast_to(b1v, (B, GC))),
            "ident": ident,
        })
    return in_maps


def _run(s, in_maps, trace=False):
    from concourse import bass_utils
    key = (s,)
    if key not in _cache:
        _cache[key] = _build(s)
    nc = _cache[key]
    res = bass_utils.run_bass_kernel_spmd(
        nc, in_maps, core_ids=list(range(NC)), trace=trace)
    return res


class _Runner:
    """Persistent jitted SPMD runner: stage inputs once, call repeatedly."""

    def __init__(self, s):
        import jax
        import numpy as _np
        import concourse.mybir as mybir
        from jax.sharding import Mesh, PartitionSpec
        from jax.experimental.shard_map import shard_map
        from concourse import bass2jax

        if (s,) not in _cache:
            _cache[(s,)] = _build(s)
        nc = _cache[(s,)]
        bass2jax.install_neuronx_cc_hook()
        self.nc = nc
        in_names, out_names, out_avals, zero_outs = [], [], [], []
        for alloc in nc.m.functions[0].allocations:
            if not isinstance(alloc, mybir.MemoryLocationSet):
                continue
            name = alloc.memorylocations[0].name
            if alloc.kind == "ExternalInput":
                if nc.partition_id_tensor is not None and \
                        name == nc.partition_id_tensor.name:
                    continue
                in_names.append(name)
            elif alloc.kind == "ExternalOutput":
                out_names.append(name)
                shape = tuple(alloc.tensor_shape)
                dtype = mybir.dt.np(alloc.dtype)
                out_avals.append(jax.core.ShapedArray(shape, dtype))
                zero_outs.append(_np.zeros(shape, dtype))
        self.in_names, self.out_names = in_names, out_names
        self.out_avals, self.zero_outs = out_avals, zero_outs
        n_params, n_outs = len(in_names), len(out_names)
        all_names = in_names + out_names

        pname = nc.partition_id_tensor.name if nc.partition_id_tensor else None
        if pname is not None:
            all_names = all_names + [pname]

        def _body(*args):
            operands = list(args)
            if pname is not None:
                operands.append(bass2jax.partition_id_tensor())
            outs = bass2jax._bass_exec_p.bind(
                *operands,
                out_avals=tuple(out_avals),
                in_names=tuple(all_names),
                out_names=tuple(out_names),
                lowering_input_output_aliases=(),
                sim_require_finite=True, sim_require_nnan=True, nc=nc)
            return tuple(outs)

        devices = jax.devices()[:NC]
        self.mesh = Mesh(_np.asarray(devices), ("core",))
        in_specs = (PartitionSpec("core"),) * (n_params + n_outs)
        out_specs = (PartitionSpec("core"),) * n_outs
        self.donate = tuple(range(n_params, n_params + n_outs))
        self.fn = jax.jit(
            shard_map(_body, mesh=self.mesh, in_specs=in_specs,
                      out_specs=out_specs, check_rep=False),
            donate_argnums=self.donate, keep_unused=True)
        self.jax = jax

    def stage(self, in_maps):
        import numpy as _np
        jax = self.jax
        from jax.sharding import NamedSharding, PartitionSpec
        sh = NamedSharding(self.mesh, PartitionSpec("core"))
        self.dev_in = [
            jax.device_put(_np.concatenate(
                [_np.asarray(m[n]) for m in in_maps], axis=0), sh)
            for n in self.in_names]
        jax.block_until_ready(self.dev_in)

    def _zeros(self):
        import numpy as _np
        jax = self.jax
        from jax.sharding import NamedSharding, PartitionSpec
        sh = NamedSharding(self.mesh, PartitionSpec("core"))
        z = [jax.device_put(_np.zeros((NC * t.shape[0], *t.shape[1:]), t.dtype), sh)
             for t in self.zero_outs]
        jax.block_until_ready(z)
        return z

    def run(self, timeit=False):
        import time as _time
        jax = self.jax
        z = self._zeros()
        t0 = _time.perf_counter()
        out = self.fn(*self.dev_in, *z)
        jax.block_until_ready(out)
        dt = _time.perf_counter() - t0
        return out, dt

    def results(self, out):
        import numpy as _np
        res = []
        for c in range(NC):
            d = {}
            for i, n in enumerate(self.out_names):
                a = _np.asarray(out[i]).reshape(NC, *self.out_avals[i].shape)
                d[n] = a[c]
            res.append(d)
        return res


def _assemble(s, results):
    output = np.empty((s, B, H), np.float32)
    h_n = np.empty((2, B, H), np.float32)
    c_n = np.empty((2, B, H), np.float32)
    for c in range(NC):
        sl = slice(c * HC, (c + 1) * HC)
        output[:, :, sl] = results[c]["out_c"]
        h_n[:, :, sl] = results[c]["hn_c"]
        c_n[:, :, sl] = results[c]["cn_c"]
    return output, h_n, c_n


def kernel(x, Wih0, bih0, Whh0, bhh0, Wih1, bih1, Whh1, bhh1):
    x = np.asarray(x, np.float32)
    in_maps = _shard_inputs(S, x, Wih0, bih0, Whh0, bhh0,
                            Wih1, bih1, Whh1, bhh1)
    res = _run(S, in_maps)
    return _assemble(S, res.results)
